# revision 10
# baseline (speedup 1.0000x reference)
"""DecoderRNN (teacher-forced GRU decoder + vocab projection + log_softmax).

Device program (identical on all 8 cores; per-core inputs differ only in
values — each core receives its own V/8 slice of W_out.T):
  phase A: h0 = tanh(encoder_final @ W_enc.T + b_enc)
  phase B: gi[t,b,:] = emb[trg[b,t-1]] @ W_ih.T + (b_ih + b_hh)   (bulk, all t)
  phase C: 64-step GRU scan (replicated), float32r matmuls
  phase D: logits slice = H @ W_out.T[:, slice] (float32r), online max/sumexp
           stats per row, per-chunk top-8 candidates
Host: assemble slices, global logsumexp, logp = logits - lse, argmax/mask/loss.

float32r (~13-bit mantissa TF32-like matmul) was validated against the fixed
reference seed: 0 argmax flips, logp error <= 4e-5.
"""

import numpy as np

import concourse.bass as bass
import concourse.mybir as mybir
import concourse.tile as tile
from concourse.tile import add_dep_helper
from concourse.bass_utils import run_bass_kernel_spmd

F32 = mybir.dt.float32
F32R = mybir.dt.float32r
I32 = mybir.dt.int32
U32 = mybir.dt.uint32

import os as _os
B, T, E, D, ENC, V = 32, 64, 512, 1024, 1024, 32000
T = int(_os.environ.get("DBG_T", T))
NCORES = 8
VS = V // NCORES          # 4000 vocab slice per core
VC = 500                  # vocab chunk (psum bank)
NVC = VS // VC            # 8 chunks per core
ROWS = T * B              # 2048
RT = ROWS // 128          # 16 row tiles
KD = D // 128             # 8 contraction chunks over D
KE = E // 128             # 4 contraction chunks over E
G = 3 * D                 # 3072 gate width
NG = G // 512             # 6 gate chunks

_CACHED = {}
LAST_EXEC_NS = None


def _build_program():
    nc = bass.Bass()

    # ---- DRAM tensors -------------------------------------------------
    WhhT = nc.dram_tensor("WhhT", [D, G], F32R, kind="ExternalInput")
    WihT = nc.dram_tensor("WihT", [E, G], F32R, kind="ExternalInput")
    WencT = nc.dram_tensor("WencT", [ENC, D], F32R, kind="ExternalInput")
    encfT = nc.dram_tensor("encfT", [ENC, B], F32R, kind="ExternalInput")
    embt = nc.dram_tensor("embt", [V, E], F32R, kind="ExternalInput")
    biasg = nc.dram_tensor("biasg", [1, G], F32R, kind="ExternalInput")
    bence = nc.dram_tensor("bence", [1, D], F32R, kind="ExternalInput")
    idxe = nc.dram_tensor("idxe", [RT, 128], I32, kind="ExternalInput")
    id32 = nc.dram_tensor("id32", [32, 32], F32R, kind="ExternalInput")
    id128 = nc.dram_tensor("id128", [128, 128], F32, kind="ExternalInput")
    ones1 = nc.dram_tensor("ones1", [1, 128], F32R, kind="ExternalInput")
    WoT = nc.dram_tensor("WoT", [D, VS], F32R, kind="ExternalInput")

    lgraw = nc.dram_tensor("lgraw", [ROWS, VS], F32, kind="ExternalOutput")
    mstat = nc.dram_tensor("mstat", [RT, 128], F32, kind="ExternalOutput")
    sstat = nc.dram_tensor("sstat", [RT, 128], F32, kind="ExternalOutput")
    cmax8 = nc.dram_tensor("cmax8", [RT, NVC, 128, 8], F32, kind="ExternalOutput")
    cidx8 = nc.dram_tensor("cidx8", [RT, NVC, 128, 8], U32, kind="ExternalOutput")

    gi_buf = nc.dram_tensor("gi_buf", [ROWS, G], F32R)       # internal
    # HT layout: [kc, d_in_chunk, t, b]
    HT_buf = nc.dram_tensor("HT_buf", [KD, 128, T, B], F32R)  # internal

    predrain_targets = []
    _pd_last = {}

    def _pd(bi, last_only=False):
        if last_only:
            _pd_last[getattr(bi.ins, "engine", "x")] = bi
        else:
            predrain_targets.append(bi)
        return bi

    with tile.TileContext(nc) as tc:
        with (
            tc.tile_pool(name="wpool", bufs=1) as wpool,
            tc.tile_pool(name="cpool", bufs=1) as cpool,
        ):
            # persistent constants
            id32_s = cpool.tile([32, 32], F32R, tag="id32")
            nc.sync.dma_start(id32_s[:], id32[:, :])
            id128_s = cpool.tile([128, 128], F32, tag="id128")
            nc.sync.dma_start(id128_s[:], id128[:, :])
            ones_s = cpool.tile([1, 128], F32R, tag="ones")
            nc.sync.dma_start(ones_s[:], ones1[:, :])
            biasg_s = cpool.tile([1, G], F32R, tag="biasg")
            nc.sync.dma_start(biasg_s[:], biasg[:, :])
            bence_s = cpool.tile([1, D], F32R, tag="bence")
            nc.sync.dma_start(bence_s[:], bence[:, :])

            # ---------------- phase A: h0 -----------------------------
            with (
                tc.tile_pool(name="apool", bufs=1) as ap_,
                tc.tile_pool(name="apsum", bufs=1, space="PSUM") as aps,
            ):
                wenc_s = ap_.tile([128, KD * D], F32R, tag="wenc")
                nc.sync.dma_start(
                    wenc_s[:].rearrange("p (k d) -> p k d", k=KD),
                    bass.AP(WencT[:, :].tensor, 0, [[D, 128], [128 * D, KD], [1, D]]),
                )
                encf_s = ap_.tile([128, KD * B], F32R, tag="encf")
                nc.sync.dma_start(
                    encf_s[:].rearrange("p (k b) -> p k b", k=KD),
                    bass.AP(encfT[:, :].tensor, 0, [[B, 128], [128 * B, KD], [1, B]]),
                )
                h_nat = cpool.tile([32, D], F32, tag="h_nat")
                for j in range(2):
                    hp = aps.tile([32, 512], F32, tag="h0p")
                    for kc in range(KD):
                        nc.tensor.matmul(
                            hp[:],
                            encf_s[:, kc * B:(kc + 1) * B],
                            wenc_s[:, kc * D + j * 512: kc * D + (j + 1) * 512],
                            start=(kc == 0), stop=False,
                        )
                    nc.tensor.matmul(
                        hp[:], ones_s[:, :32],
                        bence_s[:, j * 512:(j + 1) * 512],
                        start=False, stop=True,
                    )
                    nc.scalar.activation(
                        h_nat[:, j * 512:(j + 1) * 512], hp[:],
                        mybir.ActivationFunctionType.Tanh,
                    )
                # transpose h0 -> hT state
                hT_r = cpool.tile([128, KD * 32], F32R, tag="hT")
                htp = aps.tile([128, KD * 32], F32, tag="htp")
                for kc in range(KD):
                    nc.tensor.transpose(
                        htp[:, kc * 32:(kc + 1) * 32],
                        h_nat[:, kc * 128:(kc + 1) * 128],
                        id32_s[:].bitcast(F32),
                    )
                nc.vector.tensor_copy(hT_r[:], htp[:])

            # ---------------- phase B: gi precompute ------------------
            with (
                tc.tile_pool(name="bpool", bufs=2) as bp,
                tc.tile_pool(name="bpool1", bufs=1) as bp1,
                tc.tile_pool(name="bpsum", bufs=1, space="PSUM") as bps,
            ):
                wih_s = bp1.tile([128, KE * G], F32R, tag="wih")
                nc.sync.dma_start(
                    wih_s[:].rearrange("p (k g) -> p k g", k=KE),
                    bass.AP(WihT[:, :].tensor, 0, [[G, 128], [128 * G, KE], [1, G]]),
                )
                for rt in range(RT):
                    idx_s = bp.tile([128, 1], I32, tag="idx")
                    nc.sync.dma_start(
                        idx_s[:],
                        bass.AP(idxe[:, :].tensor, rt * 128, [[1, 128], [1, 1]]))
                    xs_s = bp.tile([128, E], F32R, tag="xs")
                    nc.gpsimd.indirect_dma_start(
                        out=xs_s[:], out_offset=None,
                        in_=embt[:, :],
                        in_offset=bass.IndirectOffsetOnAxis(ap=idx_s[:, :1], axis=0),
                    )
                    if rt == 0:
                        nc.gpsimd.memset(xs_s[0:32, :].bitcast(F32), 0.0)
                    # transpose xs -> xsT chunks
                    xtp = bps.tile([128, KE * 128], F32, tag="xtp")
                    for kc in range(KE):
                        nc.tensor.transpose(
                            xtp[:, kc * 128:(kc + 1) * 128],
                            xs_s[:, kc * 128:(kc + 1) * 128].bitcast(F32),
                            id128_s[:],
                        )
                    xsT = bp.tile([128, KE * 128], F32R, tag="xsT")
                    nc.vector.tensor_copy(xsT[:], xtp[:])
                    gi_s = bp.tile([128, G], F32R, tag="gi_s")
                    for j in range(NG):
                        gp = bps.tile([128, 512], F32, tag="gp")
                        for kc in range(KE):
                            nc.tensor.matmul(
                                gp[:],
                                xsT[:, kc * 128:(kc + 1) * 128],
                                wih_s[:, kc * G + j * 512: kc * G + (j + 1) * 512],
                                start=(kc == 0), stop=False,
                            )
                        nc.tensor.matmul(
                            gp[:], ones_s[:, :128],
                            biasg_s[:, j * 512:(j + 1) * 512],
                            start=False, stop=True,
                        )
                        nc.any.tensor_copy(gi_s[:, j * 512:(j + 1) * 512], gp[:])
                    nc.sync.dma_start(gi_buf[rt * 128:(rt + 1) * 128, :], gi_s[:])

            # ---------------- phase C: GRU scan -----------------------
            with (
                tc.tile_pool(name="cpoolS", bufs=2) as sp,
                tc.tile_pool(name="cpoolS1", bufs=1) as sp1,
                tc.tile_pool(name="cpsum", bufs=1, space="PSUM") as cps,
            ):
                whh_s = sp1.tile([128, KD * G], F32R, tag="whh")
                nc.sync.dma_start(
                    whh_s[:].rearrange("p (k g) -> p k g", k=KD),
                    bass.AP(WhhT[:, :].tensor, 0, [[G, 128], [128 * G, KD], [1, G]]),
                )
                h_prev = h_nat  # fp32 [32, D] from phase A
                for t in range(T):
                    gi_t = sp.tile([32, G], F32R, tag="gi_t")
                    nc.sync.dma_start(gi_t[:], gi_buf[t * 32:(t + 1) * 32, :])
                    # gates r,z: psum = gh + gi     (4 chunks of 512)
                    rs = sp1.tile([32, D], F32, tag="rs")
                    zs = sp1.tile([32, D], F32, tag="zs")
                    zn = sp1.tile([32, D], F32, tag="zn")   # 1 - z
                    grz = []
                    for j in range(4):
                        gp = cps.tile([32, 512], F32, tag=f"grz{j}")
                        for kc in range(KD):
                            nc.tensor.matmul(
                                gp[:],
                                hT_r[:, kc * 32:(kc + 1) * 32],
                                whh_s[:, kc * G + j * 512: kc * G + (j + 1) * 512],
                                start=(kc == 0), stop=False,
                            )
                        nc.tensor.matmul(
                            gp[:], id32_s[:],
                            gi_t[:, j * 512:(j + 1) * 512],
                            start=False, stop=True,
                        )
                        grz.append(gp)
                        dst = rs if j < 2 else zs
                        off = (j % 2) * 512
                        nc.scalar.activation(
                            dst[:, off:off + 512], gp[:],
                            mybir.ActivationFunctionType.Sigmoid,
                        )
                        if j >= 2:
                            nc.scalar.activation(
                                zn[:, off:off + 512], gp[:],
                                mybir.ActivationFunctionType.Sigmoid, scale=-1.0,
                            )
                    # n gate: psum = gh_n only (2 chunks)
                    n_s = sp1.tile([32, D], F32, tag="n_s")
                    h_new = sp.tile([32, D], F32, tag="h_new")
                    for j in range(2):
                        gp = cps.tile([32, 512], F32, tag=f"gn{j}")
                        for kc in range(KD):
                            nc.tensor.matmul(
                                gp[:],
                                hT_r[:, kc * 32:(kc + 1) * 32],
                                whh_s[:, kc * G + (4 + j) * 512: kc * G + (5 + j) * 512],
                                start=(kc == 0), stop=(kc == KD - 1),
                            )
                        off = j * 512
                        t1 = sp1.tile([32, 512], F32, tag=f"t1{j}")
                        nc.vector.tensor_tensor(
                            out=t1[:], in0=gp[:], in1=rs[:, off:off + 512],
                            op=mybir.AluOpType.mult,
                        )
                        t2 = sp1.tile([32, 512], F32, tag=f"t2{j}")
                        nc.vector.tensor_tensor(
                            out=t2[:], in0=t1[:],
                            in1=gi_t[:, 2048 + off: 2048 + off + 512].bitcast(F32),
                            op=mybir.AluOpType.add,
                        )
                        nc.scalar.activation(
                            n_s[:, off:off + 512], t2[:],
                            mybir.ActivationFunctionType.Tanh,
                        )
                        # blend: h' = zn*n + z*h
                        a1 = sp1.tile([32, 512], F32, tag=f"a1{j}")
                        nc.vector.tensor_tensor(
                            out=a1[:], in0=n_s[:, off:off + 512],
                            in1=zn[:, off:off + 512], op=mybir.AluOpType.mult,
                        )
                        a2 = sp1.tile([32, 512], F32, tag=f"a2{j}")
                        nc.vector.tensor_tensor(
                            out=a2[:], in0=h_prev[:, off:off + 512],
                            in1=zs[:, off:off + 512], op=mybir.AluOpType.mult,
                        )
                        nc.vector.tensor_tensor(
                            out=h_new[:, off:off + 512], in0=a1[:], in1=a2[:],
                            op=mybir.AluOpType.add,
                        )
                    # transpose h_new -> hT state + store to HT_buf
                    htp2 = cps.tile([128, KD * 32], F32, tag="htp2")
                    for kc in range(KD):
                        nc.tensor.transpose(
                            htp2[:, kc * 32:(kc + 1) * 32],
                            h_new[:, kc * 128:(kc + 1) * 128],
                            id32_s[:].bitcast(F32),
                        )
                    hT_r = sp.tile([128, KD * 32], F32R, tag="hT_roll")
                    nc.vector.tensor_copy(hT_r[:], htp2[:])
                    nc.sync.dma_start(
                        bass.AP(HT_buf[:, :, :, :].tensor, t * B,
                                [[T * B, 128], [128 * T * B, KD], [1, B]]),
                        hT_r[:].rearrange("p (k b) -> p k b", k=KD),
                    )
                    h_prev = h_new

            # ---------------- phase D: projection ---------------------
            with (
                tc.tile_pool(name="dpool", bufs=1) as dp1,
                tc.tile_pool(name="dpool2", bufs=2) as dp2,
                tc.tile_pool(name="dpool3", bufs=3) as dp3,
                tc.tile_pool(name="dpsum", bufs=2, space="PSUM") as dps,
            ):
                # all HT tiles resident: [rt][kc] -> [128, 128]
                ht_all = dp1.tile([128, RT * KD * 128], F32R, tag="ht_all")
                for rt in range(RT):
                    for kc in range(KD):
                        nc.sync.dma_start(
                            ht_all[:, (rt * KD + kc) * 128:(rt * KD + kc + 1) * 128]
                            .rearrange("p (t b) -> p t b", b=B),
                            bass.AP(HT_buf[:, :, :, :].tensor,
                                    kc * 128 * T * B + rt * 4 * B,
                                    [[T * B, 128], [B, 4], [1, B]]),
                        )
                m_run = [dp1.tile([128, 1], F32, tag=f"m{rt}", name=f"m_run{rt}") for rt in range(RT)]
                s_run = [dp1.tile([128, 1], F32, tag=f"s{rt}", name=f"s_run{rt}") for rt in range(RT)]
                scr = dp1.tile([128, VC], F32, tag="scr")
                for vc in range(NVC):
                    wv = dp2.tile([128, KD * VC], F32R, tag="wv")
                    nc.sync.dma_start(
                        wv[:].rearrange("p (k v) -> p k v", k=KD),
                        bass.AP(WoT[:, :].tensor, vc * VC,
                                [[VS, 128], [128 * VS, KD], [1, VC]]),
                    )
                    for rt in range(RT):
                        lp = dps.tile([128, VC], F32, tag="lp")
                        for kc in range(KD):
                            nc.tensor.matmul(
                                lp[:],
                                ht_all[:, (rt * KD + kc) * 128:(rt * KD + kc + 1) * 128],
                                wv[:, kc * VC:(kc + 1) * VC],
                                start=(kc == 0), stop=(kc == KD - 1),
                            )
                        stage = dp3.tile([128, VC], F32, tag="stage")
                        nc.any.tensor_copy(stage[:], lp[:])
                        _pd(nc.sync.dma_start(
                            lgraw[rt * 128:(rt + 1) * 128, vc * VC:(vc + 1) * VC],
                            stage[:],
                        ), last_only=True)
                        mx8 = dp3.tile([128, 8], F32, tag="mx8")
                        nc.vector.max(mx8[:], stage[:])
                        ix8 = dp3.tile([128, 8], U32, tag="ix8")
                        nc.vector.max_index(ix8[:], mx8[:], stage[:])
                        _pd(nc.sync.dma_start(cmax8[rt, vc, :, :], mx8[:]), last_only=True)
                        _pd(nc.sync.dma_start(cidx8[rt, vc, :, :], ix8[:]), last_only=True)
                        if vc == 0:
                            nc.vector.tensor_copy(m_run[rt][:], mx8[:, 0:1])
                            negm = dp3.tile([128, 1], F32, tag="negm")
                            nc.vector.tensor_scalar_mul(negm[:], m_run[rt][:], -1.0)
                            nc.scalar.activation(
                                scr[:], stage[:],
                                mybir.ActivationFunctionType.Exp,
                                bias=negm[:, 0:1], accum_out=s_run[rt][:],
                            )
                        else:
                            mn = dp3.tile([128, 1], F32, tag="mn")
                            nc.vector.tensor_tensor(
                                out=mn[:], in0=m_run[rt][:], in1=mx8[:, 0:1],
                                op=mybir.AluOpType.max,
                            )
                            negm = dp3.tile([128, 1], F32, tag="negm")
                            nc.vector.tensor_scalar_mul(negm[:], mn[:], -1.0)
                            sc = dp3.tile([128, 1], F32, tag="sc")
                            nc.scalar.activation(
                                sc[:], m_run[rt][:],
                                mybir.ActivationFunctionType.Exp,
                                bias=negm[:, 0:1],
                            )
                            s1 = dp3.tile([128, 1], F32, tag="s1")
                            nc.vector.tensor_tensor(
                                out=s1[:], in0=s_run[rt][:], in1=sc[:],
                                op=mybir.AluOpType.mult,
                            )
                            s2 = dp3.tile([128, 1], F32, tag="s2")
                            nc.scalar.activation(
                                scr[:], stage[:],
                                mybir.ActivationFunctionType.Exp,
                                bias=negm[:, 0:1], accum_out=s2[:],
                            )
                            nc.vector.tensor_tensor(
                                out=s_run[rt][:], in0=s1[:], in1=s2[:],
                                op=mybir.AluOpType.add,
                            )
                            _pd(nc.vector.tensor_copy(m_run[rt][:], mn[:]), last_only=True)
                # write stats
                for rt in range(RT):
                    _pd(nc.sync.dma_start(
                        bass.AP(mstat[:, :].tensor, rt * 128, [[1, 128], [1, 1]]),
                        m_run[rt][:]))
                    _pd(nc.sync.dma_start(
                        bass.AP(sstat[:, :].tensor, rt * 128, [[1, 128], [1, 1]]),
                        s_run[rt][:]))

            # pre-drain: let SyncE observe outstanding procs one at a time so
            # the final Tile drain stays within the 1-wait ISA budget.
            targets = [] if _os.environ.get("NO_PREDRAIN") else (
                predrain_targets + list(_pd_last.values()))
            seen = set()
            for bi in targets:
                if bi.ins.name in seen:
                    continue
                seen.add(bi.ins.name)
                nnop = nc.sync.nop()
                add_dep_helper(nnop.ins, bi.ins, sync=True, reason="predrain")

    return nc


def _legalize_waits(nc):
    """Walrus/ISA budget: ~1 sem-wait per instruction (self-loading f32/f32r
    matmuls definitely; NOPs fail at 4+). Move excess waits onto same-engine
    NoOps inserted immediately before the offender."""
    import bass_rust
    f = nc.m.functions[0]
    n_added = 0
    for bb in f.blocks:
        insts = bb.instructions
        new = []
        for ins in insts:
            si = ins.sync_info
            if si is not None and len(si.on_wait) > 1:
                waits = list(si.on_wait)
                for k, w in enumerate(waits[:-1]):
                    nop = mybir.InstNoOp(
                        name=f"{ins.name}_lw{k}", engine=ins.engine,
                        ins=[], outs=[],
                        sync_info=bass_rust.SyncInfo(on_wait=[w], on_update=[]),
                    )
                    new.append(nop)
                    n_added += 1
                ins.sync_info = bass_rust.SyncInfo(
                    on_wait=[waits[-1]], on_update=list(si.on_update))
            new.append(ins)
        insts[:] = new
    return n_added


def _audit_waits(nc):
    """Return list of (name, type, nwaits) that exceed the per-inst budget."""
    bad = []
    f = nc.m.functions[0]
    for bb in f.blocks:
        for ins in bb.instructions:
            si = ins.sync_info
            if si is None:
                continue
            nw = str(si).count("SyncWait(")
            tn = type(ins).__name__
            lim = 1
            if nw > lim:
                bad.append((ins.name, tn, nw))
    return bad


def _marshal(inputs):
    f32 = np.float32
    enc_f = np.asarray(inputs["encoder_final"], f32)[0]          # [B, ENC]
    trg = np.asarray(inputs["trg_var"]).astype(np.int64)          # [B, T]
    emb = np.asarray(inputs["emb_table"], f32)
    W_ih = np.asarray(inputs["W_ih"], f32)
    W_hh = np.asarray(inputs["W_hh"], f32)
    b_ih = np.asarray(inputs["b_ih"], f32)
    b_hh = np.asarray(inputs["b_hh"], f32)
    W_enc = np.asarray(inputs["W_enc"], f32)
    b_enc = np.asarray(inputs["b_enc"], f32)
    W_out = np.asarray(inputs["W_out"], f32)

    base = {
        "WhhT": np.ascontiguousarray(W_hh.T),
        "WihT": np.ascontiguousarray(W_ih.T),
        "WencT": np.ascontiguousarray(W_enc.T),
        "encfT": np.ascontiguousarray(enc_f.T),
        "embt": np.ascontiguousarray(emb),
        "biasg": (b_ih + b_hh).reshape(1, G),
        "bence": b_enc.reshape(1, D),
        "id32": np.eye(32, dtype=f32),
        "id128": np.eye(128, dtype=f32),
        "ones1": np.ones((1, 128), f32),
    }
    # xs row (t, b) -> emb row trg[b, t-1]; t=0 rows are zeroed on device
    idx = np.zeros((ROWS,), np.int32)
    tt, bb = np.meshgrid(np.arange(T), np.arange(B), indexing="ij")
    rows = tt.ravel() * B + bb.ravel()
    src = np.where(tt.ravel() >= 1, trg[bb.ravel(), np.maximum(tt.ravel() - 1, 0)], 0)
    idx[rows] = src.astype(np.int32)
    base["idxe"] = idx.reshape(RT, 128)

    WoutT = np.ascontiguousarray(W_out.T)                         # [D, V]
    in_maps = []
    for c in range(NCORES):
        m = dict(base)
        m["WoT"] = np.ascontiguousarray(WoutT[:, c * VS:(c + 1) * VS])
        in_maps.append(m)
    return in_maps, trg


def kernel(**inputs):
    if "nc" not in _CACHED:
        nc = _build_program()
        _legalize_waits(nc)
        bad = _audit_waits(nc)
        if bad:
            raise RuntimeError(f"wait-budget audit failed: {bad[:10]}")
        _CACHED["nc"] = nc
    nc = _CACHED["nc"]

    in_maps, trg = _marshal(inputs)
    res = run_bass_kernel_spmd(
        nc, in_maps, core_ids=list(range(NCORES)),
        trace=bool(_os.environ.get("BASS_PROFILE")),
    )
    global LAST_EXEC_NS
    LAST_EXEC_NS = res.exec_time_ns
    outs = res.results

    # ---- host: assemble + normalize + argmax + mask + loss ------------
    lg = np.concatenate([outs[c]["lgraw"] for c in range(NCORES)], axis=1)  # [2048, V]
    M_c = np.stack([outs[c]["mstat"].reshape(ROWS) for c in range(NCORES)])  # [8, 2048]
    S_c = np.stack([outs[c]["sstat"].reshape(ROWS) for c in range(NCORES)])
    M = M_c.max(0)
    S = (S_c.astype(np.float64) * np.exp(M_c.astype(np.float64) - M[None, :])).sum(0)
    lse = (M.astype(np.float64) + np.log(S)).astype(np.float32)             # [2048]
    logp = (lg - lse[:, None]).reshape(T, B, V)

    # preds: combine per-core per-chunk top-8 candidates (exact first-index
    # tie-break: smallest global index among max-valued candidates)
    vals = np.concatenate(
        [outs[c]["cmax8"].transpose(0, 2, 1, 3).reshape(ROWS, NVC * 8)
         for c in range(NCORES)], axis=1)                                    # [2048, 64*8]
    gidx = np.concatenate(
        [(outs[c]["cidx8"].astype(np.int64)
          + (np.arange(NVC) * VC)[None, :, None, None]
          + c * VS).transpose(0, 2, 1, 3).reshape(ROWS, NVC * 8)
         for c in range(NCORES)], axis=1)
    vmax = vals.max(1, keepdims=True)
    cand = np.where(vals == vmax, gidx, np.int64(1 << 60))
    preds = cand.min(1).astype(np.int32).reshape(T, B)

    # mask / loss (reference semantics)
    EOS, PAD = 2, 0
    mask = np.zeros((T, B), bool)
    cur = np.ones((B,), bool)
    for t in range(T):
        mask[t] = cur
        cur = cur & (preds[t] != EOS)
    trg_t = trg.T.astype(np.int64)                                           # [T, B]
    nll = -np.take_along_axis(
        logp.reshape(T, B, V), trg_t[..., None].astype(np.int64), axis=2)[..., 0]
    full_mask = mask & (trg_t != PAD)
    loss = np.float32(np.where(full_mask, nll, 0.0).astype(np.float32).sum() / B)

    preds_out = preds.T.astype(np.int32)                                     # [B, T]
    return logp.astype(np.float32), preds_out, mask, loss
